# revision 12
# baseline (speedup 1.0000x reference)
"""Sliding-window GQA causal self-attention (ALiBi) Trainium2 Bass kernel.

Problem: B=2, T=4096, C=1024, H=16, HKV=4 (GQA G=4), D=64, window W=512,
fused qkv projection + sliding-window attention + output projection.

Sharding: data-parallel over (batch x T/4) -> 8 cores. Each core computes
1024 query rows of one batch plus a 512-row k/v halo. No collectives.

Per-core dataflow (matmuls in float32r except PV/out-proj in bf16):
  - x arrives host-transposed; xT streamed in 256-column time slices
  - qT computed transposed into a packed layout [kv][qb*512 + g*128 + q]
    so one N=512 score matmul covers all 4 GQA heads of a kv group;
    kT computed transposed per kv head; v natural (bf16) with a ones
    column appended per (chunk, kv) block
  - scores computed TRANSPOSED directly: sT[keys, q] = kT_aug^T @ qT_aug
    per 128-key chunk (kT chunk stationary, K=67; 3 augmentation rows
    fold the ALiBi bias and left-edge -1e9 penalty) - no PE transposes
  - window mask: triangle adds on the two extreme chunks only (DVE)
  - softmax without max-subtraction (scores are N(0,~6.5); exp cannot
    overflow at <13 sigma) - shift-invariance keeps it exact
  - PV accumulates oT[65, 512] over exactly 5 key chunks; the v ones
    column makes row 64 the softmax denominator for free
  - normalization AFTER PV: reciprocal of one [1,512] row, broadcast
    across 64 partitions via a K=1 matmul, one [64,512] multiply into
    bf16 attnT (even g direct, odd g staged through an SBUF DMA for the
    partition shift); deferred one kv step so the PE never stalls on it
  - out = attnT.T @ wo (bf16) per q-block, interleaved with attention
"""

import math
from contextlib import ExitStack

import numpy as np
import ml_dtypes

import concourse.bass as bass
from concourse import bacc
import concourse.mybir as mybir
import concourse.tile as tile
from concourse.bass_utils import run_bass_kernel_spmd

F32 = mybir.dt.float32
F32R = mybir.dt.float32r
BF16 = mybir.dt.bfloat16

B, T, C = 2, 4096, 1024
H, HKV, G, D = 16, 4, 4, 64
W = 512
NCORES = 8
RT = 1024              # own query rows per core
KR = RT + W            # k/v slab rows (512 halo + 1024 own)
NQB = RT // 128        # 8 q-blocks of 128
NKC = KR // 128        # 12 k-chunks of 128
SCALE = D ** -0.5      # 0.125, exact power of two
NEG = -1e9
KCOL0 = C              # wqkv col offset of k
VCOL0 = C + HKV * D    # wqkv col offset of v
VW = 65                # v block width: 64 features + ones column


def alibi_slopes(n_head: int) -> np.ndarray:
    def slopes_power_of_2(n):
        start = 2.0 ** (-(2.0 ** (-(math.log2(n) - 3))))
        return [start * start ** i for i in range(n)]

    if float(math.log2(n_head)).is_integer():
        s = slopes_power_of_2(n_head)
    else:
        closest = 2 ** math.floor(math.log2(n_head))
        s = slopes_power_of_2(closest)
        s2 = slopes_power_of_2(2 * closest)
        s += s2[0::2][: n_head - closest]
    return np.array(s, dtype=np.float32)


def build_nc(loop: int = 1) -> bacc.Bacc:
    nc = bacc.Bacc("TRN2", target_bir_lowering=False)

    xs = nc.dram_tensor("xs", [C, KR], F32R, kind="ExternalInput")  # host-transposed
    wqkv = nc.dram_tensor("wqkv", [C, C + 2 * HKV * D], F32R, kind="ExternalInput")
    wob = nc.dram_tensor("wob", [C, C], BF16, kind="ExternalInput")
    qaug = nc.dram_tensor("qaug", [HKV, 3, G * RT], F32R, kind="ExternalInput")
    kaug = nc.dram_tensor("kaug", [3, KR], F32R, kind="ExternalInput")
    m0q4 = nc.dram_tensor("m0q4", [128, 512], BF16, kind="ExternalInput")
    w4q4 = nc.dram_tensor("w4q4", [128, 512], F32, kind="ExternalInput")
    out = nc.dram_tensor("out", [RT, C], F32, kind="ExternalOutput")

    Exp = mybir.ActivationFunctionType.Exp

    with tile.TileContext(nc) as tc:
      for _rep in range(loop):
        with ExitStack() as ctx:
            persist = ctx.enter_context(tc.tile_pool(name="persist", bufs=1))

            # packed qT per kv: rows 0:64 data, 64:67 aug; col = qb*512+g*128+q
            qTs = [persist.tile([128, NQB * 512], F32R, name=f"qT{kv}")
                   for kv in range(HKV)]
            kTs = [persist.tile([128, KR], F32R, name=f"kT{kv}")
                   for kv in range(HKV)]
            # v natural bf16, kc-major: [t-in-chunk, kc*(4*65) + kv*65 + (d|ones)]
            vsl = persist.tile([128, NKC * HKV * VW], BF16)
            wo_sb = persist.tile([128, 8 * 1024], BF16)
            m0_sb = persist.tile([128, 512], BF16)
            w4_sb = persist.tile([128, 512], F32)

            # ones column of every (kc, kv) v block
            vones = bass.AP(tensor=vsl.tensor, offset=vsl.offset + 64,
                            ap=[list(vsl.ap[0]), [HKV * VW, NKC], [VW, HKV]])
            nc.vector.memset(vones, 1.0)


            xTp = ctx.enter_context(tc.tile_pool(name="xTp", bufs=2))
            wqp = ctx.enter_context(tc.tile_pool(name="wqp", bufs=1))
            stp = ctx.enter_context(tc.tile_pool(name="stp", bufs=3))
            pTp = ctx.enter_context(tc.tile_pool(name="pTp", bufs=6))
            atp = ctx.enter_context(tc.tile_pool(name="atp", bufs=2))
            obp = ctx.enter_context(tc.tile_pool(name="obp", bufs=2))
            sgp = ctx.enter_context(tc.tile_pool(name="sgp", bufs=3))
            rsp = ctx.enter_context(tc.tile_pool(name="rsp", bufs=2))
            bcp = ctx.enter_context(tc.tile_pool(name="bcp", bufs=2))
            psA = ctx.enter_context(tc.tile_pool(name="psA", bufs=2, space="PSUM"))
            psS = ctx.enter_context(tc.tile_pool(name="psS", bufs=3, space="PSUM"))
            psO = ctx.enter_context(tc.tile_pool(name="psO", bufs=2, space="PSUM"))

            # stationary weights: wkv[cc-major k|v], wq_all[cc-major q cols],
            # loaded with single big 3D-AP DMAs on the ACT HWDGE queue so the
            # xT stream on the SP queue is never blocked behind them.
            wkv = wqp.tile([128, 8 * 512], F32R)
            wq_all = wqp.tile([128, 8 * 1024], F32R)
            CW = C + 2 * HKV * D  # wqkv row stride

            def load_weights_head():
                s = wqkv[0:1, KCOL0:KCOL0 + 1]
                nc.scalar.dma_start(wkv, bass.AP(
                    tensor=s.tensor, offset=s.offset,
                    ap=[[CW, 128], [128 * CW, 8], [1, 512]]))

            def load_weights_tail():
                s = wqkv[0:1, 0:1]
                nc.scalar.dma_start(wq_all, bass.AP(
                    tensor=s.tensor, offset=s.offset,
                    ap=[[CW, 128], [128 * CW, 8], [1, 1024]]))
                nc.scalar.dma_start(m0_sb, m0q4[:, :])
                nc.scalar.dma_start(w4_sb, w4q4[:, :])
                for kv in range(HKV):
                    nc.scalar.dma_start(qTs[kv][64:67, :], qaug[kv, :, :])
                    nc.scalar.dma_start(kTs[kv][64:67, :], kaug[:, :])

            def load_slice(ts):
                xTt = xTp.tile([128, 8 * 256], F32R, tag="xts")
                s = xs[0:1, ts * 256:ts * 256 + 1]
                nc.sync.dma_start(xTt, bass.AP(
                    tensor=s.tensor, offset=s.offset,
                    ap=[[KR, 128], [128 * KR, 8], [1, 256]]))
                return xTt

            def proj_slice(ts, xTt):
                t0 = ts * 256
                # k projection: fi=0 -> (kv0,kv1), fi=1 -> (kv2,kv3)
                for fi in range(2):
                    pst = psA.tile([128, 512], F32, tag="ps")
                    ps = pst[:, 0:256]
                    for cc in range(8):
                        nc.tensor.matmul(
                            ps,
                            lhsT=wkv[:, cc * 512 + fi * 128:cc * 512 + (fi + 1) * 128],
                            rhs=xTt[:, cc * 256:(cc + 1) * 256],
                            start=(cc == 0), stop=(cc == 7))
                    kv0, kv1 = 2 * fi, 2 * fi + 1
                    nc.scalar.copy(kTs[kv0][0:64, t0:t0 + 256], ps[0:64, :])
                    st = stp.tile([128, 256], F32R, tag="st")
                    nc.vector.tensor_copy(st[64:128, :], ps[64:128, :])
                    nc.sync.dma_start(kTs[kv1][0:64, t0:t0 + 256], st[64:128, :])
                # v projection: two 128-t chunks per slice
                for tki in range(2):
                    kc = ts * 2 + tki
                    psvt = psA.tile([128, 512], F32, tag="ps")
                    psv = psvt[:, 0:256]
                    for cc in range(8):
                        nc.tensor.matmul(
                            psv,
                            lhsT=xTt[:, cc * 256 + tki * 128:cc * 256 + (tki + 1) * 128],
                            rhs=wkv[:, cc * 512 + 256:cc * 512 + 512],
                            start=(cc == 0), stop=(cc == 7))
                    vdst = bass.AP(tensor=vsl.tensor,
                                   offset=vsl.offset + kc * HKV * VW,
                                   ap=[list(vsl.ap[0]), [VW, HKV], [1, 64]])
                    nc.scalar.copy(vdst, psv.rearrange("p (a b) -> p a b", b=64))
                # q projection (own rows only)
                if ts >= 2:
                    toff = t0 - 512
                    qb0 = toff // 128
                    for kv in range(HKV):
                        for fi in range(2):
                            pst = psA.tile([128, 512], F32, tag="ps")
                            ps = pst[:, 0:256]
                            for cc in range(8):
                                nc.tensor.matmul(
                                    ps,
                                    lhsT=wq_all[:, cc * 1024 + kv * 256 + fi * 128:
                                                cc * 1024 + kv * 256 + (fi + 1) * 128],
                                    rhs=xTt[:, cc * 256:(cc + 1) * 256],
                                    start=(cc == 0), stop=(cc == 7))
                            ge, go = 2 * fi, 2 * fi + 1
                            qd = qTs[kv]
                            dste = bass.AP(
                                tensor=qd.tensor,
                                offset=qd.offset + qb0 * 512 + ge * 128,
                                ap=[[qd.ap[0][0], 64], [512, 2], [1, 128]])
                            nc.scalar.copy(
                                dste, ps[0:64, :].rearrange("p (a b) -> p a b", b=128))
                            st = stp.tile([128, 256], F32R, tag="st")
                            nc.vector.tensor_copy(st[64:128, :], ps[64:128, :])
                            dsto = bass.AP(
                                tensor=qd.tensor,
                                offset=qd.offset + qb0 * 512 + go * 128,
                                ap=[[qd.ap[0][0], 64], [512, 2], [1, 128]])
                            nc.sync.dma_start(dsto, st[64:128, :].rearrange(
                                "p (a b) -> p a b", b=128))

            def pair(src, off):
                s64 = src[0:64, :]
                return bass.AP(tensor=s64.tensor, offset=s64.offset + off,
                               ap=[list(s64.ap[0]), [256, 2], [1, 128]])

            def emit_scale(at, po, rs, kv):
                # broadcast 1/s across 64 partitions with a replicating DMA
                bcs = bcp.tile([64, 512], F32, tag="bcs")
                r64 = rs[64:65, :]
                brd = bass.AP(tensor=r64.tensor, offset=r64.offset,
                              ap=[list(r64.ap[0]), [0, 64], [1, 512]])
                nc.scalar.dma_start(bcs, brd)
                # even g -> attnT rows 0:64 directly; odd g staged via DMA
                nc.vector.tensor_mul(at[0:64, kv * 256:kv * 256 + 256],
                                     pair(po, 0), pair(bcs, 0))
                sg = sgp.tile([64, 256], BF16, tag="sg")
                nc.vector.tensor_mul(sg, pair(po, 128), pair(bcs, 128))
                nc.sync.dma_start(at[64:128, kv * 256:kv * 256 + 256], sg)

            pend = {}

            def flush_pending():
                # deferred (qb, kv=3) scale of the previous q-block, then its
                # output projection
                if not pend:
                    return
                at, po, rs, qb = pend["at"], pend["po"], pend["rs"], pend["qb"]
                emit_scale(at, po, rs, 3)
                ob = obp.tile([128, 1024], F32, tag="ob")
                for ec in range(2):
                    pf = psA.tile([128, 512], F32, tag="ps")
                    for cc in range(8):
                        nc.tensor.matmul(
                            pf,
                            lhsT=at[:, cc * 128:(cc + 1) * 128],
                            rhs=wo_sb[:, cc * 1024 + ec * 512:
                                      cc * 1024 + ec * 512 + 512],
                            start=(cc == 0), stop=(cc == 7))
                    nc.vector.tensor_copy(ob[:, ec * 512:(ec + 1) * 512], pf)
                nc.sync.dma_start(out[qb * 128:(qb + 1) * 128, :], ob)
                pend.clear()

            def attn_block(qb):
                at = atp.tile([128, 8 * 128], BF16, tag="at")
                blk = {}
                for kv in range(HKV):
                    pcs = []
                    for j in range(5):
                        ck = qb + j
                        ps = psS.tile([128, 512], F32, tag="sc")
                        nc.tensor.matmul(
                            ps,
                            lhsT=kTs[kv][0:67, ck * 128:(ck + 1) * 128],
                            rhs=qTs[kv][0:67, qb * 512:(qb + 1) * 512],
                            start=True, stop=True)
                        if j == 4:
                            nc.vector.tensor_add(ps, ps, w4_sb)
                        pc = pTp.tile([128, 512], BF16, tag="pc")
                        nc.scalar.activation(pc, ps, Exp, bias=0.0)
                        if j == 0:
                            nc.gpsimd.tensor_mul(pc, pc, m0_sb)
                        pcs.append(pc)
                    # deferred scaling keeps the PE from stalling on recip
                    if kv == 0:
                        flush_pending()
                    else:
                        ppo, prs = blk[kv - 1]
                        emit_scale(at, ppo, prs, kv - 1)
                    po = psO.tile([65, 512], F32, tag="ot")
                    for j in range(5):
                        base = (qb + j) * HKV * VW + kv * VW
                        nc.tensor.matmul(po, lhsT=vsl[:, base:base + VW],
                                         rhs=pcs[j], start=(j == 0), stop=(j == 4))
                    rs = rsp.tile([65, 512], F32, tag="rs")
                    with nc.allow_low_precision(reason="fp32 out"):
                        nc.vector.reciprocal(rs[64:65, :], po[64:65, :])
                    blk[kv] = (po, rs)
                pend.update(at=at, po=blk[3][0], rs=blk[3][1], qb=qb)

            # ---------------- schedule ----------------
            xts = [load_slice(0)]
            load_weights_head()
            xts += [load_slice(1), load_slice(2)]
            load_weights_tail()
            proj_slice(0, xts[0])
            proj_slice(1, xts[1])
            proj_slice(2, xts[2])
            s = wob[0:1, 0:1]
            nc.scalar.dma_start(wo_sb, bass.AP(
                tensor=s.tensor, offset=s.offset,
                ap=[[C, 128], [128 * C, 8], [1, 1024]]))
            for ts in range(3, 6):
                xts.append(load_slice(ts))
                attn_block(2 * ts - 6)
                attn_block(2 * ts - 5)
                proj_slice(ts, xts[ts])
            attn_block(6)
            attn_block(7)
            flush_pending()

    nc.compile()
    return nc


_NC = None


def _host_inputs(x, wqkv, wo):
    slopes = alibi_slopes(H)  # head h = kv*G + g matches slopes.reshape(HKV, G)

    wqkv_s = np.array(wqkv, dtype=np.float32, copy=True)
    wqkv_s[:, :C] *= SCALE  # exact power-of-two fold of the score scale into wq

    # packed q augmentation: col = qb*512 + g*128 + q, t = qb*128 + q
    qaug = np.empty((HKV, 3, G * RT), dtype=np.float32)
    cols = np.arange(G * RT)
    col_t = (cols // 512) * 128 + (cols % 128)
    col_g = (cols % 512) // 128
    for kv in range(HKV):
        sl = slopes[kv * G + col_g]
        qaug[kv, 0] = -sl * (col_t + 512.0)
        qaug[kv, 1] = sl
        qaug[kv, 2] = 1.0

    i = np.arange(KR, dtype=np.float32)
    kaug_base = np.empty((3, KR), dtype=np.float32)
    kaug_base[0] = 1.0
    kaug_base[1] = i
    kaug_base[2] = 0.0

    # transposed-score window masks on the extreme chunks, tiled for 4 g:
    # chunk j=0: valid q < r (0/1 multiply on p); chunk j=4: valid q >= r
    r = np.arange(128)[:, None]
    q = np.arange(128)[None, :]
    m0 = np.where(q < r, 1.0, 0.0).astype(ml_dtypes.bfloat16)
    w4 = np.where(q < r, np.float32(NEG), np.float32(0.0)).astype(np.float32)
    m0q4 = np.ascontiguousarray(np.tile(m0, (1, 4)))
    w4q4 = np.ascontiguousarray(np.tile(w4, (1, 4)))

    wob = np.asarray(wo, dtype=np.float32).astype(ml_dtypes.bfloat16)

    in_maps = []
    for core in range(NCORES):
        b, qq = core // 4, core % 4
        t0 = qq * RT
        xsl = np.zeros((KR, C), dtype=np.float32)
        lo = t0 - W
        if lo < 0:
            xsl[-lo:, :] = x[b, 0:t0 + RT, :]
        else:
            xsl[:, :] = x[b, lo:t0 + RT, :]
        xsl = np.ascontiguousarray(xsl.T)
        kaug = kaug_base.copy()
        if lo < 0:
            kaug[2, :W] = NEG  # left-edge penalty kills padded keys
        in_maps.append(dict(xs=xsl, wqkv=wqkv_s, wob=wob,
                            qaug=qaug, kaug=kaug, m0q4=m0q4, w4q4=w4q4))
    return in_maps


def kernel(x, wqkv, wo):
    global _NC
    if _NC is None:
        _NC = build_nc()
    in_maps = _host_inputs(np.asarray(x), np.asarray(wqkv), np.asarray(wo))
    res = run_bass_kernel_spmd(_NC, in_maps, list(range(NCORES)))
    full = np.empty((B, T, C), dtype=np.float32)
    for core in range(NCORES):
        b, qq = core // 4, core % 4
        full[b, qq * RT:(qq + 1) * RT, :] = res.results[core]["out"]
    return full


# revision 13
# speedup vs baseline: 2.7814x; 2.7814x over previous
"""Sliding-window GQA causal self-attention (ALiBi) Trainium2 Bass kernel.

Problem: B=2, T=4096, C=1024, H=16, HKV=4 (GQA G=4), D=64, window W=512,
fused qkv projection + sliding-window attention + output projection.

Sharding: data-parallel over (batch x T/4) -> 8 cores. Each core computes
1024 query rows of one batch plus a 512-row k/v halo. No collectives.

Per-core dataflow (matmuls in float32r except PV/out-proj in bf16):
  - x arrives host-transposed; xT streamed in 256-column time slices
  - qT computed transposed into a packed layout [kv][qb*512 + g*128 + q]
    so one N=512 score matmul covers all 4 GQA heads of a kv group;
    kT computed transposed per kv head; v natural (bf16) with a ones
    column appended per (chunk, kv) block
  - scores computed TRANSPOSED directly: sT[keys, q] = kT_aug^T @ qT_aug
    per 128-key chunk (kT chunk stationary, K=67; 3 augmentation rows
    fold the ALiBi bias and left-edge -1e9 penalty) - no PE transposes
  - window mask: triangle adds on the two extreme chunks only (DVE)
  - softmax without max-subtraction (scores are N(0,~6.5); exp cannot
    overflow at <13 sigma) - shift-invariance keeps it exact
  - PV accumulates oT[65, 512] over exactly 5 key chunks; the v ones
    column makes row 64 the softmax denominator for free
  - normalization AFTER PV: reciprocal of one [1,512] row, broadcast
    across 64 partitions via a K=1 matmul, one [64,512] multiply into
    bf16 attnT (even g direct, odd g staged through an SBUF DMA for the
    partition shift); deferred one kv step so the PE never stalls on it
  - out = attnT.T @ wo (bf16) per q-block, interleaved with attention
"""

import math
from contextlib import ExitStack

import numpy as np
import ml_dtypes

import concourse.bass as bass
from concourse import bacc
import concourse.mybir as mybir
import concourse.tile as tile
from concourse.bass_utils import run_bass_kernel_spmd

F32 = mybir.dt.float32
F32R = mybir.dt.float32r
BF16 = mybir.dt.bfloat16

B, T, C = 2, 4096, 1024
H, HKV, G, D = 16, 4, 4, 64
W = 512
NCORES = 8
RT = 1024              # own query rows per core
KR = RT + W            # k/v slab rows (512 halo + 1024 own)
NQB = RT // 128        # 8 q-blocks of 128
NKC = KR // 128        # 12 k-chunks of 128
SCALE = D ** -0.5      # 0.125, exact power of two
NEG = -1e9
KCOL0 = C              # wqkv col offset of k
VCOL0 = C + HKV * D    # wqkv col offset of v
VW = 65                # v block width: 64 features + ones column


def alibi_slopes(n_head: int) -> np.ndarray:
    def slopes_power_of_2(n):
        start = 2.0 ** (-(2.0 ** (-(math.log2(n) - 3))))
        return [start * start ** i for i in range(n)]

    if float(math.log2(n_head)).is_integer():
        s = slopes_power_of_2(n_head)
    else:
        closest = 2 ** math.floor(math.log2(n_head))
        s = slopes_power_of_2(closest)
        s2 = slopes_power_of_2(2 * closest)
        s += s2[0::2][: n_head - closest]
    return np.array(s, dtype=np.float32)


def build_nc(loop: int = 1) -> bacc.Bacc:
    nc = bacc.Bacc("TRN2", target_bir_lowering=False)

    xs = nc.dram_tensor("xs", [C, KR], F32R, kind="ExternalInput")  # host-transposed
    wqkv = nc.dram_tensor("wqkv", [C, C + 2 * HKV * D], F32R, kind="ExternalInput")
    wob = nc.dram_tensor("wob", [C, C], BF16, kind="ExternalInput")
    qaug = nc.dram_tensor("qaug", [HKV, 3, G * RT], F32R, kind="ExternalInput")
    kaug = nc.dram_tensor("kaug", [3, KR], F32R, kind="ExternalInput")
    m0q4 = nc.dram_tensor("m0q4", [128, 512], BF16, kind="ExternalInput")
    w4q4 = nc.dram_tensor("w4q4", [128, 512], F32, kind="ExternalInput")
    out = nc.dram_tensor("out", [RT, C], F32, kind="ExternalOutput")

    Exp = mybir.ActivationFunctionType.Exp

    with tile.TileContext(nc) as tc:
      for _rep in range(loop):
        with ExitStack() as ctx:
            persist = ctx.enter_context(tc.tile_pool(name="persist", bufs=1))

            # packed qT per kv: rows 0:64 data, 64:67 aug; col = qb*512+g*128+q
            qTs = [persist.tile([128, NQB * 512], F32R, name=f"qT{kv}")
                   for kv in range(HKV)]
            kTs = [persist.tile([128, KR], F32R, name=f"kT{kv}")
                   for kv in range(HKV)]
            # v natural bf16, kc-major: [t-in-chunk, kc*(4*65) + kv*65 + (d|ones)]
            vsl = persist.tile([128, NKC * HKV * VW], BF16)
            wo_sb = persist.tile([128, 8 * 1024], BF16)
            m0_sb = persist.tile([128, 512], BF16)
            w4_sb = persist.tile([128, 512], F32)

            # ones column of every (kc, kv) v block
            vones = bass.AP(tensor=vsl.tensor, offset=vsl.offset + 64,
                            ap=[list(vsl.ap[0]), [HKV * VW, NKC], [VW, HKV]])
            nc.vector.memset(vones, 1.0)


            xTp = ctx.enter_context(tc.tile_pool(name="xTp", bufs=2))
            wqp = ctx.enter_context(tc.tile_pool(name="wqp", bufs=1))
            stp = ctx.enter_context(tc.tile_pool(name="stp", bufs=3))
            pTp = ctx.enter_context(tc.tile_pool(name="pTp", bufs=6))
            atp = ctx.enter_context(tc.tile_pool(name="atp", bufs=2))
            obp = ctx.enter_context(tc.tile_pool(name="obp", bufs=2))
            sgp = ctx.enter_context(tc.tile_pool(name="sgp", bufs=3))
            rsp = ctx.enter_context(tc.tile_pool(name="rsp", bufs=2))
            bcp = ctx.enter_context(tc.tile_pool(name="bcp", bufs=2))
            psA = ctx.enter_context(tc.tile_pool(name="psA", bufs=2, space="PSUM"))
            psS = ctx.enter_context(tc.tile_pool(name="psS", bufs=3, space="PSUM"))
            psO = ctx.enter_context(tc.tile_pool(name="psO", bufs=2, space="PSUM"))

            # stationary weights: wkv[cc-major k|v], wq_all[cc-major q cols],
            # loaded with single big 3D-AP DMAs on the ACT HWDGE queue so the
            # xT stream on the SP queue is never blocked behind them.
            wkv = wqp.tile([128, 8 * 512], F32R)
            wq_all = wqp.tile([128, 8 * 1024], F32R)
            CW = C + 2 * HKV * D  # wqkv row stride

            def load_weights_head():
                for cc in range(8):
                    nc.scalar.dma_start(wkv[:, cc * 512:(cc + 1) * 512],
                                        wqkv[cc * 128:(cc + 1) * 128, KCOL0:KCOL0 + 512])

            def load_weights_tail():
                for cc in range(8):
                    nc.scalar.dma_start(wq_all[:, cc * 1024:(cc + 1) * 1024],
                                        wqkv[cc * 128:(cc + 1) * 128, 0:1024])
                nc.scalar.dma_start(m0_sb, m0q4[:, :])
                nc.scalar.dma_start(w4_sb, w4q4[:, :])
                for kv in range(HKV):
                    nc.scalar.dma_start(qTs[kv][64:67, :], qaug[kv, :, :])
                    nc.scalar.dma_start(kTs[kv][64:67, :], kaug[:, :])

            def load_slice(ts):
                xTt = xTp.tile([128, 8 * 256], F32R, tag="xts")
                for cc in range(8):
                    nc.sync.dma_start(
                        xTt[:, cc * 256:(cc + 1) * 256],
                        xs[cc * 128:(cc + 1) * 128, ts * 256:(ts + 1) * 256])
                return xTt

            def proj_slice(ts, xTt):
                t0 = ts * 256
                # k projection: fi=0 -> (kv0,kv1), fi=1 -> (kv2,kv3)
                for fi in range(2):
                    pst = psA.tile([128, 512], F32, tag="ps")
                    ps = pst[:, 0:256]
                    for cc in range(8):
                        nc.tensor.matmul(
                            ps,
                            lhsT=wkv[:, cc * 512 + fi * 128:cc * 512 + (fi + 1) * 128],
                            rhs=xTt[:, cc * 256:(cc + 1) * 256],
                            start=(cc == 0), stop=(cc == 7))
                    kv0, kv1 = 2 * fi, 2 * fi + 1
                    nc.scalar.copy(kTs[kv0][0:64, t0:t0 + 256], ps[0:64, :])
                    st = stp.tile([128, 256], F32R, tag="st")
                    nc.vector.tensor_copy(st[64:128, :], ps[64:128, :])
                    nc.sync.dma_start(kTs[kv1][0:64, t0:t0 + 256], st[64:128, :])
                # v projection: two 128-t chunks per slice
                for tki in range(2):
                    kc = ts * 2 + tki
                    psvt = psA.tile([128, 512], F32, tag="ps")
                    psv = psvt[:, 0:256]
                    for cc in range(8):
                        nc.tensor.matmul(
                            psv,
                            lhsT=xTt[:, cc * 256 + tki * 128:cc * 256 + (tki + 1) * 128],
                            rhs=wkv[:, cc * 512 + 256:cc * 512 + 512],
                            start=(cc == 0), stop=(cc == 7))
                    vdst = bass.AP(tensor=vsl.tensor,
                                   offset=vsl.offset + kc * HKV * VW,
                                   ap=[list(vsl.ap[0]), [VW, HKV], [1, 64]])
                    nc.scalar.copy(vdst, psv.rearrange("p (a b) -> p a b", b=64))
                # q projection (own rows only)
                if ts >= 2:
                    toff = t0 - 512
                    qb0 = toff // 128
                    for kv in range(HKV):
                        for fi in range(2):
                            pst = psA.tile([128, 512], F32, tag="ps")
                            ps = pst[:, 0:256]
                            for cc in range(8):
                                nc.tensor.matmul(
                                    ps,
                                    lhsT=wq_all[:, cc * 1024 + kv * 256 + fi * 128:
                                                cc * 1024 + kv * 256 + (fi + 1) * 128],
                                    rhs=xTt[:, cc * 256:(cc + 1) * 256],
                                    start=(cc == 0), stop=(cc == 7))
                            ge, go = 2 * fi, 2 * fi + 1
                            qd = qTs[kv]
                            dste = bass.AP(
                                tensor=qd.tensor,
                                offset=qd.offset + qb0 * 512 + ge * 128,
                                ap=[[qd.ap[0][0], 64], [512, 2], [1, 128]])
                            nc.scalar.copy(
                                dste, ps[0:64, :].rearrange("p (a b) -> p a b", b=128))
                            st = stp.tile([128, 256], F32R, tag="st")
                            nc.vector.tensor_copy(st[64:128, :], ps[64:128, :])
                            dsto = bass.AP(
                                tensor=qd.tensor,
                                offset=qd.offset + qb0 * 512 + go * 128,
                                ap=[[qd.ap[0][0], 64], [512, 2], [1, 128]])
                            nc.sync.dma_start(dsto, st[64:128, :].rearrange(
                                "p (a b) -> p a b", b=128))

            def pair(src, off):
                s64 = src[0:64, :]
                return bass.AP(tensor=s64.tensor, offset=s64.offset + off,
                               ap=[list(s64.ap[0]), [256, 2], [1, 128]])

            def emit_scale(at, po, rs, kv):
                # broadcast 1/s across 64 partitions with a replicating DMA
                bcs = bcp.tile([64, 512], F32, tag="bcs")
                r64 = rs[64:65, :]
                brd = bass.AP(tensor=r64.tensor, offset=r64.offset,
                              ap=[list(r64.ap[0]), [0, 64], [1, 512]])
                nc.scalar.dma_start(bcs, brd)
                # even g -> attnT rows 0:64 directly; odd g staged via DMA
                nc.vector.tensor_mul(at[0:64, kv * 256:kv * 256 + 256],
                                     pair(po, 0), pair(bcs, 0))
                sg = sgp.tile([64, 256], BF16, tag="sg")
                nc.vector.tensor_mul(sg, pair(po, 128), pair(bcs, 128))
                nc.sync.dma_start(at[64:128, kv * 256:kv * 256 + 256], sg)

            pend = {}

            def flush_pending():
                # deferred (qb, kv=3) scale of the previous q-block, then its
                # output projection
                if not pend:
                    return
                at, po, rs, qb = pend["at"], pend["po"], pend["rs"], pend["qb"]
                emit_scale(at, po, rs, 3)
                ob = obp.tile([128, 1024], F32, tag="ob")
                for ec in range(2):
                    pf = psA.tile([128, 512], F32, tag="ps")
                    for cc in range(8):
                        nc.tensor.matmul(
                            pf,
                            lhsT=at[:, cc * 128:(cc + 1) * 128],
                            rhs=wo_sb[:, cc * 1024 + ec * 512:
                                      cc * 1024 + ec * 512 + 512],
                            start=(cc == 0), stop=(cc == 7))
                    nc.vector.tensor_copy(ob[:, ec * 512:(ec + 1) * 512], pf)
                nc.sync.dma_start(out[qb * 128:(qb + 1) * 128, :], ob)
                pend.clear()

            def attn_block(qb):
                at = atp.tile([128, 8 * 128], BF16, tag="at")
                blk = {}
                for kv in range(HKV):
                    pcs = []
                    for j in range(5):
                        ck = qb + j
                        ps = psS.tile([128, 512], F32, tag="sc")
                        nc.tensor.matmul(
                            ps,
                            lhsT=kTs[kv][0:67, ck * 128:(ck + 1) * 128],
                            rhs=qTs[kv][0:67, qb * 512:(qb + 1) * 512],
                            start=True, stop=True)
                        if j == 4:
                            nc.vector.tensor_add(ps, ps, w4_sb)
                        pc = pTp.tile([128, 512], BF16, tag="pc")
                        nc.scalar.activation(pc, ps, Exp, bias=0.0)
                        if j == 0:
                            nc.gpsimd.tensor_mul(pc, pc, m0_sb)
                        pcs.append(pc)
                    # deferred scaling keeps the PE from stalling on recip
                    if kv == 0:
                        flush_pending()
                    else:
                        ppo, prs = blk[kv - 1]
                        emit_scale(at, ppo, prs, kv - 1)
                    po = psO.tile([65, 512], F32, tag="ot")
                    for j in range(5):
                        base = (qb + j) * HKV * VW + kv * VW
                        nc.tensor.matmul(po, lhsT=vsl[:, base:base + VW],
                                         rhs=pcs[j], start=(j == 0), stop=(j == 4))
                    rs = rsp.tile([65, 512], F32, tag="rs")
                    with nc.allow_low_precision(reason="fp32 out"):
                        nc.vector.reciprocal(rs[64:65, :], po[64:65, :])
                    blk[kv] = (po, rs)
                pend.update(at=at, po=blk[3][0], rs=blk[3][1], qb=qb)

            # ---------------- schedule ----------------
            xts = [load_slice(0)]
            load_weights_head()
            xts += [load_slice(1), load_slice(2)]
            load_weights_tail()
            proj_slice(0, xts[0])
            proj_slice(1, xts[1])
            proj_slice(2, xts[2])
            for cc in range(8):
                nc.scalar.dma_start(wo_sb[:, cc * 1024:(cc + 1) * 1024],
                                    wob[cc * 128:(cc + 1) * 128, :])
            for ts in range(3, 6):
                xts.append(load_slice(ts))
                attn_block(2 * ts - 6)
                attn_block(2 * ts - 5)
                proj_slice(ts, xts[ts])
            attn_block(6)
            attn_block(7)
            flush_pending()

    nc.compile()
    return nc


_NC = None


def _host_inputs(x, wqkv, wo):
    slopes = alibi_slopes(H)  # head h = kv*G + g matches slopes.reshape(HKV, G)

    wqkv_s = np.array(wqkv, dtype=np.float32, copy=True)
    wqkv_s[:, :C] *= SCALE  # exact power-of-two fold of the score scale into wq

    # packed q augmentation: col = qb*512 + g*128 + q, t = qb*128 + q
    qaug = np.empty((HKV, 3, G * RT), dtype=np.float32)
    cols = np.arange(G * RT)
    col_t = (cols // 512) * 128 + (cols % 128)
    col_g = (cols % 512) // 128
    for kv in range(HKV):
        sl = slopes[kv * G + col_g]
        qaug[kv, 0] = -sl * (col_t + 512.0)
        qaug[kv, 1] = sl
        qaug[kv, 2] = 1.0

    i = np.arange(KR, dtype=np.float32)
    kaug_base = np.empty((3, KR), dtype=np.float32)
    kaug_base[0] = 1.0
    kaug_base[1] = i
    kaug_base[2] = 0.0

    # transposed-score window masks on the extreme chunks, tiled for 4 g:
    # chunk j=0: valid q < r (0/1 multiply on p); chunk j=4: valid q >= r
    r = np.arange(128)[:, None]
    q = np.arange(128)[None, :]
    m0 = np.where(q < r, 1.0, 0.0).astype(ml_dtypes.bfloat16)
    w4 = np.where(q < r, np.float32(NEG), np.float32(0.0)).astype(np.float32)
    m0q4 = np.ascontiguousarray(np.tile(m0, (1, 4)))
    w4q4 = np.ascontiguousarray(np.tile(w4, (1, 4)))

    wob = np.asarray(wo, dtype=np.float32).astype(ml_dtypes.bfloat16)

    in_maps = []
    for core in range(NCORES):
        b, qq = core // 4, core % 4
        t0 = qq * RT
        xsl = np.zeros((KR, C), dtype=np.float32)
        lo = t0 - W
        if lo < 0:
            xsl[-lo:, :] = x[b, 0:t0 + RT, :]
        else:
            xsl[:, :] = x[b, lo:t0 + RT, :]
        xsl = np.ascontiguousarray(xsl.T)
        kaug = kaug_base.copy()
        if lo < 0:
            kaug[2, :W] = NEG  # left-edge penalty kills padded keys
        in_maps.append(dict(xs=xsl, wqkv=wqkv_s, wob=wob,
                            qaug=qaug, kaug=kaug, m0q4=m0q4, w4q4=w4q4))
    return in_maps


def kernel(x, wqkv, wo):
    global _NC
    if _NC is None:
        _NC = build_nc()
    in_maps = _host_inputs(np.asarray(x), np.asarray(wqkv), np.asarray(wo))
    res = run_bass_kernel_spmd(_NC, in_maps, list(range(NCORES)))
    full = np.empty((B, T, C), dtype=np.float32)
    for core in range(NCORES):
        b, qq = core // 4, core % 4
        full[b, qq * RT:(qq + 1) * RT, :] = res.results[core]["out"]
    return full
